# revision 11
# baseline (speedup 1.0000x reference)
"""MoE feed-forward (top-2 routing, 8 experts) on 8 Trainium2 NeuronCores.

Expert-parallel sharding: host computes the (tiny) router + argsort
permutation exactly as the reference does, gathers each expert's token
chunk (pre-transposed to [D, C]), and sends chunk e + expert e's
weights to core e. Each core runs a dense FFN: y = gelu(x @ w1) @ w2,
scaled by the per-row gate weight. Host then inverts the permutation
and sums the top-2 contributions.

Device kernel (per core, SPMD):
  - mm1 in bf16: hT[f,c] = w1[d,f]^T xT, accumulated over 8 d-tiles.
  - gelu on ScalarE, output bf16.
  - mm2 in bf16 (w2 resident in SBUF): y[c,d] += hT[f,c]^T @ w2[f,d],
    accumulated over all 32 f-tiles in PSUM.
  - gate scale applied during the PSUM->SBUF drain on ScalarE.

The kernel is PE-bound: 2048 matmuls x 512 rows = 1.05M PE cycles is
the bf16 floor (437us at 2.4 GHz, ~530us when the chip is in the P0
power state at ~2.0 GHz). The remaining slack is startup, the HAM
cold-clock ramp, and the final drain:
  - a few dummy matmuls on a memset tile start the HAM activity window
    while the first input DMAs are still in flight, so the real stream
    begins at full clock instead of paying ~4us of half-rate matmuls;
  - the first c-block pair's xT tiles load in 512-col halves (ch=0
    halves first) so the first f-tile group can start ~1.5us earlier;
  - the very last PSUM group splits its 32-matmul accumulation into
    two 16-matmul halves: the first half drains (with gate scale)
    behind the second half's matmuls, and a single DVE
    scalar_tensor_tensor merges half B (scaled, read straight from
    PSUM) into the drained half A, so only one short op + the output
    DMA remain exposed after the last matmul;
  - output DMAs alternate between the two hardware DGE rings (SP and
    Activation) instead of serializing on one.
"""

import os
import sys

# Register the CPU jax backend alongside whatever platform is configured,
# so the router can be computed on CPU (bit-exact with a CPU-evaluated
# reference). A no-op if jax is already initialized or cpu is present.
if "jax" not in sys.modules:
    _jp = os.environ.get("JAX_PLATFORMS", "")
    if _jp and "cpu" not in _jp.split(","):
        os.environ["JAX_PLATFORMS"] = _jp + ",cpu"

import numpy as np

# Static problem config
B, T, D, FF, E, TOP_K = 4, 2048, 1024, 4096, 8, 2
N = B * T                    # 8192 tokens
NE = N * TOP_K               # 16384 expanded rows
C_PER = NE // E              # 2048 rows per core / expert chunk
P = 128
FT = FF // P                 # 32 f-tiles
DT = D // P                  # 8 d-tiles
NCB = 4                      # c-blocks per core
CB = C_PER // NCB            # 512 tokens per c-block
NCT = C_PER // P             # 16 c-tiles of 128
NWARM = 4                    # HAM warm-up matmuls before the real stream

_cache = {}


MM1_BF16 = True  # mm1 operand dtype: bf16 (faster DMA) vs float32r


def _build_program(act_name="Gelu"):
    import concourse.mybir as mybir
    import concourse.tile as tile
    from concourse import bacc

    f32 = mybir.dt.float32
    f32r = mybir.dt.float32r
    bf16 = mybir.dt.bfloat16
    mm1dt = bf16 if MM1_BF16 else f32r
    Act = mybir.ActivationFunctionType
    Alu = mybir.AluOpType

    nc = bacc.Bacc("TRN2", num_devices=E)
    xt_d = nc.dram_tensor("xt", [D, C_PER], mm1dt, kind="ExternalInput")
    w1_d = nc.dram_tensor("w1r", [FT, P, DT, P], mm1dt, kind="ExternalInput")
    w2_d = nc.dram_tensor("w2b", [FF, D], bf16, kind="ExternalInput")
    sw_d = nc.dram_tensor("swt", [P, NCT], f32, kind="ExternalInput")
    y_d = nc.dram_tensor("y", [C_PER, D], f32, kind="ExternalOutput")

    with tile.TileContext(nc) as tc:
        with tc.tile_pool(name="sb", bufs=1) as sb, \
             tc.tile_pool(name="w1p", bufs=8) as w1p, \
             tc.tile_pool(name="yop", bufs=3) as yop, \
             tc.tile_pool(name="psh", bufs=4, space="PSUM") as psh, \
             tc.tile_pool(name="psy", bufs=4, space="PSUM") as psy:
            cpool = xtp = w2p = hp = sb

            w2t = [None] * FT
            xt = {}

            # HAM warm-up: the PE clock gate opens only after ~3.4us of
            # sustained tensor activity. Real matmuls can't start until
            # the first xT/w1 DMAs land (~3us after the framework
            # preamble), so burn that window on dummy matmuls over a
            # memset tile — by the time real data arrives the PE is at
            # full clock instead of 1.2 GHz.
            wz = sb.tile([P, CB], mm1dt, tag="wz", name="wz")
            nc.gpsimd.memset(wz, 0)
            pwarm = psy.tile([P, CB], f32, tag="psy", name="pwarm")
            for _ in range(NWARM):
                nc.tensor.matmul(pwarm, wz[:, 0:P], wz, start=True,
                                 stop=True)

            def alloc_xt(d, pair):
                t = xtp.tile([P, 2 * CB], mm1dt, tag=f"xt_{d}_{pair}",
                             name=f"xt_{d}_{pair}")
                xt[(d, pair)] = t
                return t

            def load_xt_full(d, pair, eng):
                # one tile spans BOTH c-blocks of a pair: their columns
                # are adjacent in C, so each partition line is one 2 KB
                # contiguous DMA line
                t = alloc_xt(d, pair)
                eng.dma_start(
                    out=t,
                    in_=xt_d.ap()[d * P:(d + 1) * P,
                                  pair * 2 * CB:(pair + 1) * 2 * CB])

            def load_xt_half(d, pair, half, eng):
                t = xt.get((d, pair))
                if t is None:
                    t = alloc_xt(d, pair)
                c0 = pair * 2 * CB + half * CB
                eng.dma_start(
                    out=t[:, half * CB:(half + 1) * CB],
                    in_=xt_d.ap()[d * P:(d + 1) * P, c0:c0 + CB])

            # Startup: all four DMA paths stage the first f-tile
            # group's working set in the order the first matmul group
            # consumes it. The two hardware DGE rings (SP=sync,
            # Act=scalar; low first-byte latency) carry the critical
            # head — w1's d0 block (32 KB), xT ch=0 halves of d0..d5,
            # the rest of w1[ft0] and a preloaded w1[ft1]. The
            # software-DGE queues on GpSimd and DVE (~2 us fixed
            # completion latency but full SDMA bandwidth) carry the
            # later-deadline chunks concurrently: d6/d7 ch=0 and all
            # ch=1 halves.
            t0 = alloc_xt(0, 0)
            nc.scalar.dma_start(out=t0[:, 0:CB], in_=xt_d.ap()[0:P, 0:CB])
            w1t0a = w1p.tile([P, DT // 2, P], mm1dt, tag="w1a", name="w1a")
            nc.sync.dma_start(out=w1t0a[:, 0:1], in_=w1_d.ap()[0, :, 0:1])
            load_xt_half(1, 0, 0, nc.sync)
            load_xt_half(2, 0, 0, nc.scalar)
            load_xt_half(6, 0, 0, nc.gpsimd)
            load_xt_half(7, 0, 0, nc.gpsimd)
            nc.sync.dma_start(out=w1t0a[:, 1:DT // 2],
                              in_=w1_d.ap()[0, :, 1:DT // 2])
            w1t0b = w1p.tile([P, DT // 2, P], mm1dt, tag="w1b", name="w1b")
            nc.scalar.dma_start(out=w1t0b, in_=w1_d.ap()[0, :, DT // 2:DT])
            load_xt_half(3, 0, 0, nc.sync)
            load_xt_half(4, 0, 0, nc.scalar)
            load_xt_half(5, 0, 0, nc.sync)
            for d in range(DT):
                load_xt_half(d, 0, 1, nc.gpsimd)
            w1t_pre = w1p.tile([P, DT, P], mm1dt, tag="w1", name="w1t_pre")
            nc.scalar.dma_start(out=w1t_pre, in_=w1_d.ap()[1])
            swt = cpool.tile([P, NCT], f32, tag="swt")
            nc.scalar.dma_start(out=swt, in_=sw_d.ap())

            def load_w2(ft):
                # full-width tiles: 2 KB DMA lines per partition (the
                # d-half split had 1 KB lines, doubling packet count on
                # the DGE queue)
                t = w2p.tile([P, D], bf16, tag=f"w2_{ft}",
                             name=f"w2_{ft}")
                nc.sync.dma_start(
                    out=t, in_=w2_d.ap()[ft * P:(ft + 1) * P, :])
                w2t[ft] = t

            # c-blocks are processed in PAIRS for mm1: each streamed w1
            # tile feeds 16 matmuls (two 512-col groups) instead of 8,
            # halving both the w1 consumption cadence and total w1
            # re-streaming (32 MB -> 16 MB).
            ht = {}
            ndrain = 0
            for pair in range(NCB // 2):
                cbs = (2 * pair, 2 * pair + 1)
                for ft in range(FT):
                    if pair == 0 and ft == 0:
                        def w1ap(d, a=w1t0a, b=w1t0b):
                            t = a if d < DT // 2 else b
                            return t[:, d % (DT // 2), :]
                    elif pair == 0 and ft == 1:
                        def w1ap(d, t=w1t_pre):
                            return t[:, d, :]
                    else:
                        # alternate the w1 stream across both HW rings
                        w1t = w1p.tile([P, DT, P], mm1dt, tag="w1",
                                       name="w1t")
                        eng = nc.scalar if ft % 2 == 0 else nc.sync
                        eng.dma_start(out=w1t, in_=w1_d.ap()[ft])

                        def w1ap(d, t=w1t):
                            return t[:, d, :]
                    if pair == 0:
                        load_w2(ft)
                    for ch, cb in enumerate(cbs):
                        hps = psh.tile([P, CB], f32, tag="psh", name="hps")
                        for d in range(DT):
                            nc.tensor.matmul(
                                hps, w1ap(d),
                                xt[(d, pair)][:, ch * CB:(ch + 1) * CB],
                                start=(d == 0), stop=(d == DT - 1))
                        h_t = hp.tile([P, CB], bf16, tag=f"h_{ch}_{ft}",
                                      name=f"h_{ch}_{ft}")
                        nc.scalar.activation(h_t, hps,
                                             getattr(Act, act_name))
                        ht[(ch, ft)] = h_t
                # mm2 per c-block of the pair: db-outer so the db=1
                # groups run last — their w2 half (pair 0) streams
                # while the db=0 groups compute
                for ch, cb in enumerate(cbs):
                    for db in range(2):
                        d0 = db * (D // 2)
                        for ct in range(CB // P):
                            g = cb * (CB // P) + ct
                            last = (cb == NCB - 1 and db == 1
                                    and ct == CB // P - 1)
                            if last:
                                # split the final accumulation so half A
                                # drains behind half B's matmuls and only
                                # one DVE merge + the DMA stay exposed
                                ypsA = psy.tile([P, D // 2], f32,
                                                tag="psy", name="ypsA")
                                for ft in range(FT // 2):
                                    nc.tensor.matmul(
                                        ypsA,
                                        ht[(ch, ft)][:, ct * P:(ct + 1) * P],
                                        w2t[ft][:, d0:d0 + D // 2],
                                        start=(ft == 0),
                                        stop=(ft == FT // 2 - 1))
                                yoA = yop.tile([P, D // 2], f32, tag="yo",
                                               name="yoA")
                                nc.scalar.activation(
                                    yoA, ypsA, Act.Copy,
                                    scale=swt[:, g:g + 1])
                                ypsB = psy.tile([P, D // 2], f32,
                                                tag="psy", name="ypsB")
                                for ft in range(FT // 2, FT):
                                    nc.tensor.matmul(
                                        ypsB,
                                        ht[(ch, ft)][:, ct * P:(ct + 1) * P],
                                        w2t[ft][:, d0:d0 + D // 2],
                                        start=(ft == FT // 2),
                                        stop=(ft == FT - 1))
                                yo = yop.tile([P, D // 2], f32, tag="yo",
                                              name="yoS")
                                # half-A merge first so its output DMA
                                # launches while DVE finishes half B
                                half = D // 4
                                for q, eng in ((0, nc.sync),
                                               (1, nc.scalar)):
                                    q0 = q * half
                                    nc.vector.scalar_tensor_tensor(
                                        yo[:, q0:q0 + half],
                                        ypsB[:, q0:q0 + half],
                                        swt[:, g:g + 1],
                                        yoA[:, q0:q0 + half],
                                        Alu.mult, Alu.add)
                                    eng.dma_start(
                                        out=y_d.ap()[
                                            g * P:(g + 1) * P,
                                            d0 + q0:d0 + q0 + half],
                                        in_=yo[:, q0:q0 + half])
                                continue
                            yps = psy.tile([P, D // 2], f32, tag="psy",
                                           name="yps")
                            for ft in range(FT):
                                nc.tensor.matmul(
                                    yps,
                                    ht[(ch, ft)][:, ct * P:(ct + 1) * P],
                                    w2t[ft][:, d0:d0 + D // 2],
                                    start=(ft == 0), stop=(ft == FT - 1))
                            yo = yop.tile([P, D // 2], f32, tag="yo",
                                          name="yo")
                            nc.scalar.activation(yo, yps,
                                                 Act.Copy,
                                                 scale=swt[:, g:g + 1])
                            # alternate output DMAs across the two DGE
                            # rings
                            eng = nc.sync if ndrain % 2 == 0 else nc.scalar
                            ndrain += 1
                            eng.dma_start(
                                out=y_d.ap()[g * P:(g + 1) * P,
                                             d0:d0 + D // 2],
                                in_=yo)
                            # prefetch the next pair's xT tiles inside
                            # the long mm2 phase on the software-DGE
                            # queue — both HW rings stay free for the
                            # w1/w2 streams and y writebacks
                            if (ch == 0 and db == 0 and ct == 0
                                    and pair + 1 < NCB // 2):
                                for d in range(DT):
                                    load_xt_full(d, pair + 1, nc.gpsimd)
    nc.compile()
    return nc


def _get_program():
    if "nc" not in _cache:
        _cache["nc"] = _build_program()
    return _cache["nc"]


def _routing(xf, router_w):
    """Replicate the reference gating bit-exactly where it matters.

    Returns (rev, sw, sort_idx). The top-k *selection* must match the
    reference exactly (it is discrete); we therefore compute the router
    logits with jax when available, mirroring reference.py. The softmax
    and sort bookkeeping is continuous or exactly replicable in numpy.
    """
    try:
        import jax
        import jax.numpy as jnp

        def _gate():
            logits = jnp.asarray(xf) @ jnp.asarray(router_w).T
            return jax.lax.top_k(logits, TOP_K)

        try:
            cpu = jax.devices("cpu")[0]
            with jax.default_device(cpu):
                tv, ti = _gate()
        except Exception:
            tv, ti = _gate()
        topv = np.asarray(tv, dtype=np.float32)
        topi = np.asarray(ti)
    except Exception:
        logits = xf @ router_w.T
        # top-2 with jax tie-breaking (lower index wins)
        i0 = np.argmax(logits, axis=-1)
        v0 = np.take_along_axis(logits, i0[:, None], axis=-1)[:, 0]
        masked = logits.copy()
        np.put_along_axis(masked, i0[:, None], -np.inf, axis=-1)
        i1 = np.argmax(masked, axis=-1)
        v1 = np.take_along_axis(logits, i1[:, None], axis=-1)[:, 0]
        topi = np.stack([i0, i1], axis=-1)
        topv = np.stack([v0, v1], axis=-1).astype(np.float32)

    # softmax over the two gate logits, float32
    m = topv.max(axis=-1, keepdims=True)
    e = np.exp(topv - m, dtype=np.float32)
    topw = (e / e.sum(axis=-1, keepdims=True)).astype(np.float32)

    idx_flat = topi.reshape(-1)
    w_flat = topw.reshape(-1)
    # stable argsort of integer keys is uniquely determined by the keys
    sort_idx = np.argsort(idx_flat, kind="stable")
    src = np.repeat(np.arange(N), TOP_K)
    rev = src[sort_idx]
    sw = w_flat[sort_idx]
    return rev, sw, sort_idx


def _ensure_axon_hooks():
    """Make `antenv.axon_hooks` importable so run_bass_kernel_spmd's
    trace path degrades gracefully (or works, if the axon boot shim is
    available) instead of crashing on ImportError."""
    try:
        import antenv.axon_hooks  # noqa: F401
        return
    except ImportError:
        pass
    import sys
    import types
    mod = types.ModuleType("antenv.axon_hooks")
    state = {"hook": None}
    mod.set_axon_ntff_profile_hook = lambda h: state.update(hook=h)
    mod.get_axon_ntff_profile_hook = lambda: state["hook"]
    try:
        import antenv
        sys.modules["antenv.axon_hooks"] = mod
        antenv.axon_hooks = mod
    except ImportError:
        return
    try:
        from trn_agent_boot.trn_boot import _ntff_profile_via_ctypes
        h = _ntff_profile_via_ctypes("/opt/axon/libaxon_pjrt.so")
        if h is not None:
            mod.set_axon_ntff_profile_hook(h)
            import concourse.bass_utils as bu
            bu.upload_artifacts = lambda tmpdir: "local://" + str(tmpdir)
    except Exception:
        pass


def kernel(x, router_w, w1, w2):
    import ml_dtypes
    from concourse import bass_utils
    _ensure_axon_hooks()

    xf = np.ascontiguousarray(x.reshape(-1, D), dtype=np.float32)
    rev, sw, sort_idx = _routing(xf, router_w)

    nc = _get_program()

    in_maps = []
    for e in range(E):
        rows = rev[e * C_PER:(e + 1) * C_PER]
        xct = np.ascontiguousarray(xf[rows].T)
        w1r = np.ascontiguousarray(
            w1[e].reshape(DT, P, FT, P).transpose(2, 1, 0, 3))
        if MM1_BF16:
            xct = xct.astype(ml_dtypes.bfloat16)
            w1r = w1r.astype(ml_dtypes.bfloat16)
        w2b = np.ascontiguousarray(w2[e].astype(ml_dtypes.bfloat16))
        swt = np.ascontiguousarray(
            sw[e * C_PER:(e + 1) * C_PER].reshape(NCT, P).T)
        in_maps.append({"xt": xct, "w1r": w1r, "w2b": w2b, "swt": swt})

    r = bass_utils.run_bass_kernel_spmd(nc, in_maps, core_ids=list(range(E)))
    _cache["last_result"] = r

    y_sorted = np.empty((NE, D), dtype=np.float32)
    for e in range(E):
        y_sorted[e * C_PER:(e + 1) * C_PER] = r.results[e]["y"]

    # invert the permutation and combine the top-2 contributions
    y_expanded = np.empty_like(y_sorted)
    y_expanded[sort_idx] = y_sorted
    out = y_expanded.reshape(N, TOP_K, D).sum(axis=1)
    return out.reshape(B, T, D)


# revision 16
# speedup vs baseline: 1.0046x; 1.0046x over previous
"""MoE feed-forward (top-2 routing, 8 experts) on 8 Trainium2 NeuronCores.

Expert-parallel sharding: host computes the (tiny) router + argsort
permutation exactly as the reference does, gathers each expert's token
chunk (pre-transposed to [D, C]), and sends chunk e + expert e's
weights to core e. Each core runs a dense FFN: y = gelu(x @ w1) @ w2,
scaled by the per-row gate weight. Host then inverts the permutation
and sums the top-2 contributions.

Device kernel (per core, SPMD):
  - mm1 in bf16: hT[f,c] = w1[d,f]^T xT, accumulated over 8 d-tiles.
  - gelu on ScalarE, output bf16.
  - mm2 in bf16 (w2 resident in SBUF): y[c,d] += hT[f,c]^T @ w2[f,d],
    accumulated over all 32 f-tiles in PSUM.
  - gate scale applied during the PSUM->SBUF drain on ScalarE.

The kernel is PE-bound: 2048 matmuls x 512 rows = 1.05M PE cycles is
the bf16 floor (437us at 2.4 GHz, ~530us when the chip is in the P0
power state at ~2.0 GHz). The remaining slack is startup, the HAM
cold-clock ramp, and the final drain:
  - a few dummy matmuls on a memset tile start the HAM activity window
    while the first input DMAs are still in flight, so the real stream
    begins at full clock instead of paying ~4us of half-rate matmuls;
  - the first c-block pair's xT tiles load in 512-col halves (ch=0
    halves first) so the first f-tile group can start ~1.5us earlier;
  - the very last PSUM group splits its 32-matmul accumulation into
    two 16-matmul halves: the first half drains (with gate scale)
    behind the second half's matmuls, and a single DVE
    scalar_tensor_tensor merges half B (scaled, read straight from
    PSUM) into the drained half A, so only one short op + the output
    DMA remain exposed after the last matmul;
  - output DMAs alternate between the two hardware DGE rings (SP and
    Activation) instead of serializing on one.
"""

import os
import sys

# Register the CPU jax backend alongside whatever platform is configured,
# so the router can be computed on CPU (bit-exact with a CPU-evaluated
# reference). A no-op if jax is already initialized or cpu is present.
if "jax" not in sys.modules:
    _jp = os.environ.get("JAX_PLATFORMS", "")
    if _jp and "cpu" not in _jp.split(","):
        os.environ["JAX_PLATFORMS"] = _jp + ",cpu"

import numpy as np

# Static problem config
B, T, D, FF, E, TOP_K = 4, 2048, 1024, 4096, 8, 2
N = B * T                    # 8192 tokens
NE = N * TOP_K               # 16384 expanded rows
C_PER = NE // E              # 2048 rows per core / expert chunk
P = 128
FT = FF // P                 # 32 f-tiles
DT = D // P                  # 8 d-tiles
NCB = 4                      # c-blocks per core
CB = C_PER // NCB            # 512 tokens per c-block
NCT = C_PER // P             # 16 c-tiles of 128
NWARM = 6                    # HAM warm-up matmuls before the real stream

_cache = {}


MM1_BF16 = True  # mm1 operand dtype: bf16 (faster DMA) vs float32r


def _build_program(act_name="Gelu"):
    import concourse.mybir as mybir
    import concourse.tile as tile
    from concourse import bacc

    f32 = mybir.dt.float32
    f32r = mybir.dt.float32r
    bf16 = mybir.dt.bfloat16
    mm1dt = bf16 if MM1_BF16 else f32r
    Act = mybir.ActivationFunctionType
    Alu = mybir.AluOpType

    nc = bacc.Bacc("TRN2", num_devices=E)
    xt_d = nc.dram_tensor("xt", [D, C_PER], mm1dt, kind="ExternalInput")
    w1_d = nc.dram_tensor("w1r", [FT, P, DT, P], mm1dt, kind="ExternalInput")
    w2_d = nc.dram_tensor("w2b", [FF, D], bf16, kind="ExternalInput")
    sw_d = nc.dram_tensor("swt", [P, NCT], f32, kind="ExternalInput")
    y_d = nc.dram_tensor("y", [C_PER, D], f32, kind="ExternalOutput")

    with tile.TileContext(nc) as tc:
        with tc.tile_pool(name="sb", bufs=1) as sb, \
             tc.tile_pool(name="w1p", bufs=8) as w1p, \
             tc.tile_pool(name="yop", bufs=3) as yop, \
             tc.tile_pool(name="psh", bufs=4, space="PSUM") as psh, \
             tc.tile_pool(name="psy", bufs=4, space="PSUM") as psy:
            cpool = xtp = w2p = hp = sb

            w2t = [None] * FT
            xt = {}

            # HAM warm-up: the PE clock gate opens only after ~3.4us of
            # sustained tensor activity. Real matmuls can't start until
            # the first xT/w1 DMAs land (~3us after the framework
            # preamble), so burn that window on dummy matmuls over a
            # memset tile — by the time real data arrives the PE is at
            # full clock instead of 1.2 GHz.
            wz = sb.tile([P, CB], mm1dt, tag="wz", name="wz")
            nc.gpsimd.memset(wz, 0)
            pwarm = psy.tile([P, CB], f32, tag="psy", name="pwarm")
            for _ in range(NWARM):
                nc.tensor.matmul(pwarm, wz[:, 0:P], wz, start=True,
                                 stop=True)

            def alloc_xt(d, pair):
                t = xtp.tile([P, 2 * CB], mm1dt, tag=f"xt_{d}_{pair}",
                             name=f"xt_{d}_{pair}")
                xt[(d, pair)] = t
                return t

            def load_xt_full(d, pair, eng):
                # one tile spans BOTH c-blocks of a pair: their columns
                # are adjacent in C, so each partition line is one 2 KB
                # contiguous DMA line
                t = alloc_xt(d, pair)
                eng.dma_start(
                    out=t,
                    in_=xt_d.ap()[d * P:(d + 1) * P,
                                  pair * 2 * CB:(pair + 1) * 2 * CB])

            def load_xt_half(d, pair, half, eng):
                t = xt.get((d, pair))
                if t is None:
                    t = alloc_xt(d, pair)
                c0 = pair * 2 * CB + half * CB
                eng.dma_start(
                    out=t[:, half * CB:(half + 1) * CB],
                    in_=xt_d.ap()[d * P:(d + 1) * P, c0:c0 + CB])

            # Startup: both hardware DGE rings (SP=sync, Act=scalar)
            # stage the first f-tile group's minimal working set first:
            # ch=0 column halves of every d-tile plus the two w1 halves,
            # interleaved so the d-order of arrival matches the d-order
            # of the first matmul group. ch=1 halves follow.
            t0 = alloc_xt(0, 0)
            nc.scalar.dma_start(out=t0[:, 0:CB], in_=xt_d.ap()[0:P, 0:CB])
            w1t0a = w1p.tile([P, DT // 2, P], mm1dt, tag="w1a", name="w1a")
            nc.sync.dma_start(out=w1t0a, in_=w1_d.ap()[0, :, 0:DT // 2])
            w1t0b = w1p.tile([P, DT // 2, P], mm1dt, tag="w1b", name="w1b")
            nc.scalar.dma_start(out=w1t0b, in_=w1_d.ap()[0, :, DT // 2:DT])
            for d in range(1, DT):
                load_xt_half(d, 0, 0, nc.sync if d % 2 else nc.scalar)
            for d in range(DT):
                load_xt_half(d, 0, 1, nc.scalar if d % 2 == 0 else nc.sync)
            swt = cpool.tile([P, NCT], f32, tag="swt")
            nc.scalar.dma_start(out=swt, in_=sw_d.ap())

            def load_w2(ft):
                # full-width tiles: 2 KB DMA lines per partition (the
                # d-half split had 1 KB lines, doubling packet count on
                # the DGE queue)
                t = w2p.tile([P, D], bf16, tag=f"w2_{ft}",
                             name=f"w2_{ft}")
                nc.sync.dma_start(
                    out=t, in_=w2_d.ap()[ft * P:(ft + 1) * P, :])
                w2t[ft] = t

            # c-blocks are processed in PAIRS for mm1: each streamed w1
            # tile feeds 16 matmuls (two 512-col groups) instead of 8,
            # halving both the w1 consumption cadence and total w1
            # re-streaming (32 MB -> 16 MB).
            ht = {}
            ndrain = 0
            for pair in range(NCB // 2):
                cbs = (2 * pair, 2 * pair + 1)
                for ft in range(FT):
                    if pair == 0 and ft == 0:
                        def w1ap(d, a=w1t0a, b=w1t0b):
                            t = a if d < DT // 2 else b
                            return t[:, d % (DT // 2), :]
                    else:
                        w1t = w1p.tile([P, DT, P], mm1dt, tag="w1",
                                       name="w1t")
                        nc.sync.dma_start(out=w1t, in_=w1_d.ap()[ft])

                        def w1ap(d, t=w1t):
                            return t[:, d, :]
                    if pair == 0:
                        load_w2(ft)
                    for ch, cb in enumerate(cbs):
                        hps = psh.tile([P, CB], f32, tag="psh", name="hps")
                        for d in range(DT):
                            nc.tensor.matmul(
                                hps, w1ap(d),
                                xt[(d, pair)][:, ch * CB:(ch + 1) * CB],
                                start=(d == 0), stop=(d == DT - 1))
                        h_t = hp.tile([P, CB], bf16, tag=f"h_{ch}_{ft}",
                                      name=f"h_{ch}_{ft}")
                        nc.scalar.activation(h_t, hps,
                                             getattr(Act, act_name))
                        ht[(ch, ft)] = h_t
                # mm2 per c-block of the pair: db-outer so the db=1
                # groups run last — their w2 half (pair 0) streams
                # while the db=0 groups compute
                for ch, cb in enumerate(cbs):
                    for db in range(2):
                        d0 = db * (D // 2)
                        for ct in range(CB // P):
                            g = cb * (CB // P) + ct
                            last = (cb == NCB - 1 and db == 1
                                    and ct == CB // P - 1)
                            if last:
                                # split the final accumulation so half A
                                # drains behind half B's matmuls and only
                                # one DVE merge + the DMA stay exposed
                                ypsA = psy.tile([P, D // 2], f32,
                                                tag="psy", name="ypsA")
                                for ft in range(FT // 2):
                                    nc.tensor.matmul(
                                        ypsA,
                                        ht[(ch, ft)][:, ct * P:(ct + 1) * P],
                                        w2t[ft][:, d0:d0 + D // 2],
                                        start=(ft == 0),
                                        stop=(ft == FT // 2 - 1))
                                yoA = yop.tile([P, D // 2], f32, tag="yo",
                                               name="yoA")
                                nc.scalar.activation(
                                    yoA, ypsA, Act.Copy,
                                    scale=swt[:, g:g + 1])
                                ypsB = psy.tile([P, D // 2], f32,
                                                tag="psy", name="ypsB")
                                for ft in range(FT // 2, FT):
                                    nc.tensor.matmul(
                                        ypsB,
                                        ht[(ch, ft)][:, ct * P:(ct + 1) * P],
                                        w2t[ft][:, d0:d0 + D // 2],
                                        start=(ft == FT // 2),
                                        stop=(ft == FT - 1))
                                yo = yop.tile([P, D // 2], f32, tag="yo",
                                              name="yoS")
                                # quarter-wise merge: each DVE quarter's
                                # output DMA launches while the next
                                # quarter is still merging, alternating
                                # DGE rings
                                quart = D // 8
                                for q in range(4):
                                    q0 = q * quart
                                    nc.vector.scalar_tensor_tensor(
                                        yo[:, q0:q0 + quart],
                                        ypsB[:, q0:q0 + quart],
                                        swt[:, g:g + 1],
                                        yoA[:, q0:q0 + quart],
                                        Alu.mult, Alu.add)
                                    eng = nc.sync if q % 2 == 0 else nc.scalar
                                    eng.dma_start(
                                        out=y_d.ap()[
                                            g * P:(g + 1) * P,
                                            d0 + q0:d0 + q0 + quart],
                                        in_=yo[:, q0:q0 + quart])
                                continue
                            yps = psy.tile([P, D // 2], f32, tag="psy",
                                           name="yps")
                            for ft in range(FT):
                                nc.tensor.matmul(
                                    yps,
                                    ht[(ch, ft)][:, ct * P:(ct + 1) * P],
                                    w2t[ft][:, d0:d0 + D // 2],
                                    start=(ft == 0), stop=(ft == FT - 1))
                            yo = yop.tile([P, D // 2], f32, tag="yo",
                                          name="yo")
                            nc.scalar.activation(yo, yps,
                                                 Act.Copy,
                                                 scale=swt[:, g:g + 1])
                            # alternate output DMAs across the two DGE
                            # rings
                            eng = nc.sync if ndrain % 2 == 0 else nc.scalar
                            ndrain += 1
                            eng.dma_start(
                                out=y_d.ap()[g * P:(g + 1) * P,
                                             d0:d0 + D // 2],
                                in_=yo)
                            # prefetch the next pair's xT tiles inside
                            # the long mm2 phase (scalar ring — sync
                            # carries the w1/w2 streams)
                            if (ch == 0 and db == 0 and ct == 0
                                    and pair + 1 < NCB // 2):
                                for d in range(DT):
                                    load_xt_full(d, pair + 1, nc.scalar)
    nc.compile()
    return nc


def _get_program():
    if "nc" not in _cache:
        _cache["nc"] = _build_program()
    return _cache["nc"]


def _routing(xf, router_w):
    """Replicate the reference gating bit-exactly where it matters.

    Returns (rev, sw, sort_idx). The top-k *selection* must match the
    reference exactly (it is discrete); we therefore compute the router
    logits with jax when available, mirroring reference.py. The softmax
    and sort bookkeeping is continuous or exactly replicable in numpy.
    """
    try:
        import jax
        import jax.numpy as jnp

        def _gate():
            logits = jnp.asarray(xf) @ jnp.asarray(router_w).T
            return jax.lax.top_k(logits, TOP_K)

        try:
            cpu = jax.devices("cpu")[0]
            with jax.default_device(cpu):
                tv, ti = _gate()
        except Exception:
            tv, ti = _gate()
        topv = np.asarray(tv, dtype=np.float32)
        topi = np.asarray(ti)
    except Exception:
        logits = xf @ router_w.T
        # top-2 with jax tie-breaking (lower index wins)
        i0 = np.argmax(logits, axis=-1)
        v0 = np.take_along_axis(logits, i0[:, None], axis=-1)[:, 0]
        masked = logits.copy()
        np.put_along_axis(masked, i0[:, None], -np.inf, axis=-1)
        i1 = np.argmax(masked, axis=-1)
        v1 = np.take_along_axis(logits, i1[:, None], axis=-1)[:, 0]
        topi = np.stack([i0, i1], axis=-1)
        topv = np.stack([v0, v1], axis=-1).astype(np.float32)

    # softmax over the two gate logits, float32
    m = topv.max(axis=-1, keepdims=True)
    e = np.exp(topv - m, dtype=np.float32)
    topw = (e / e.sum(axis=-1, keepdims=True)).astype(np.float32)

    idx_flat = topi.reshape(-1)
    w_flat = topw.reshape(-1)
    # stable argsort of integer keys is uniquely determined by the keys
    sort_idx = np.argsort(idx_flat, kind="stable")
    src = np.repeat(np.arange(N), TOP_K)
    rev = src[sort_idx]
    sw = w_flat[sort_idx]
    return rev, sw, sort_idx


def _ensure_axon_hooks():
    """Make `antenv.axon_hooks` importable so run_bass_kernel_spmd's
    trace path degrades gracefully (or works, if the axon boot shim is
    available) instead of crashing on ImportError."""
    try:
        import antenv.axon_hooks  # noqa: F401
        return
    except ImportError:
        pass
    import sys
    import types
    mod = types.ModuleType("antenv.axon_hooks")
    state = {"hook": None}
    mod.set_axon_ntff_profile_hook = lambda h: state.update(hook=h)
    mod.get_axon_ntff_profile_hook = lambda: state["hook"]
    try:
        import antenv
        sys.modules["antenv.axon_hooks"] = mod
        antenv.axon_hooks = mod
    except ImportError:
        return
    try:
        from trn_agent_boot.trn_boot import _ntff_profile_via_ctypes
        h = _ntff_profile_via_ctypes("/opt/axon/libaxon_pjrt.so")
        if h is not None:
            mod.set_axon_ntff_profile_hook(h)
            import concourse.bass_utils as bu
            bu.upload_artifacts = lambda tmpdir: "local://" + str(tmpdir)
    except Exception:
        pass


def kernel(x, router_w, w1, w2):
    import ml_dtypes
    from concourse import bass_utils
    _ensure_axon_hooks()

    xf = np.ascontiguousarray(x.reshape(-1, D), dtype=np.float32)
    rev, sw, sort_idx = _routing(xf, router_w)

    nc = _get_program()

    in_maps = []
    for e in range(E):
        rows = rev[e * C_PER:(e + 1) * C_PER]
        xct = np.ascontiguousarray(xf[rows].T)
        w1r = np.ascontiguousarray(
            w1[e].reshape(DT, P, FT, P).transpose(2, 1, 0, 3))
        if MM1_BF16:
            xct = xct.astype(ml_dtypes.bfloat16)
            w1r = w1r.astype(ml_dtypes.bfloat16)
        w2b = np.ascontiguousarray(w2[e].astype(ml_dtypes.bfloat16))
        swt = np.ascontiguousarray(
            sw[e * C_PER:(e + 1) * C_PER].reshape(NCT, P).T)
        in_maps.append({"xt": xct, "w1r": w1r, "w2b": w2b, "swt": swt})

    r = bass_utils.run_bass_kernel_spmd(nc, in_maps, core_ids=list(range(E)))
    _cache["last_result"] = r

    y_sorted = np.empty((NE, D), dtype=np.float32)
    for e in range(E):
        y_sorted[e * C_PER:(e + 1) * C_PER] = r.results[e]["y"]

    # invert the permutation and combine the top-2 contributions
    y_expanded = np.empty_like(y_sorted)
    y_expanded[sort_idx] = y_sorted
    out = y_expanded.reshape(N, TOP_K, D).sum(axis=1)
    return out.reshape(B, T, D)


# revision 17
# speedup vs baseline: 1.0058x; 1.0012x over previous
"""MoE feed-forward (top-2 routing, 8 experts) on 8 Trainium2 NeuronCores.

Expert-parallel sharding: host computes the (tiny) router + argsort
permutation exactly as the reference does, gathers each expert's token
chunk (pre-transposed to [D, C]), and sends chunk e + expert e's
weights to core e. Each core runs a dense FFN: y = gelu(x @ w1) @ w2,
scaled by the per-row gate weight. Host then inverts the permutation
and sums the top-2 contributions.

Device kernel (per core, SPMD):
  - mm1 in bf16: hT[f,c] = w1[d,f]^T xT, accumulated over 8 d-tiles.
  - gelu on ScalarE, output bf16.
  - mm2 in bf16 (w2 resident in SBUF): y[c,d] += hT[f,c]^T @ w2[f,d],
    accumulated over all 32 f-tiles in PSUM.
  - gate scale applied during the PSUM->SBUF drain on ScalarE.

The kernel is PE-bound: 2048 matmuls x 512 rows = 1.05M PE cycles is
the bf16 floor (437us at 2.4 GHz, ~530us when the chip is in the P0
power state at ~2.0 GHz). The remaining slack is startup, the HAM
cold-clock ramp, and the final drain:
  - a few dummy matmuls on a memset tile start the HAM activity window
    while the first input DMAs are still in flight, so the real stream
    begins at full clock instead of paying ~4us of half-rate matmuls;
  - the first c-block pair's xT tiles load in 512-col halves (ch=0
    halves first) so the first f-tile group can start ~1.5us earlier;
  - the very last PSUM group splits its 32-matmul accumulation into
    two 16-matmul halves: the first half drains (with gate scale)
    behind the second half's matmuls, and a single DVE
    scalar_tensor_tensor merges half B (scaled, read straight from
    PSUM) into the drained half A, so only one short op + the output
    DMA remain exposed after the last matmul;
  - output DMAs alternate between the two hardware DGE rings (SP and
    Activation) instead of serializing on one.
"""

import os
import sys

# Register the CPU jax backend alongside whatever platform is configured,
# so the router can be computed on CPU (bit-exact with a CPU-evaluated
# reference). A no-op if jax is already initialized or cpu is present.
if "jax" not in sys.modules:
    _jp = os.environ.get("JAX_PLATFORMS", "")
    if _jp and "cpu" not in _jp.split(","):
        os.environ["JAX_PLATFORMS"] = _jp + ",cpu"

import numpy as np

# Static problem config
B, T, D, FF, E, TOP_K = 4, 2048, 1024, 4096, 8, 2
N = B * T                    # 8192 tokens
NE = N * TOP_K               # 16384 expanded rows
C_PER = NE // E              # 2048 rows per core / expert chunk
P = 128
FT = FF // P                 # 32 f-tiles
DT = D // P                  # 8 d-tiles
NCB = 4                      # c-blocks per core
CB = C_PER // NCB            # 512 tokens per c-block
NCT = C_PER // P             # 16 c-tiles of 128
NWARM = 6                    # HAM warm-up matmuls before the real stream

_cache = {}


MM1_BF16 = True  # mm1 operand dtype: bf16 (faster DMA) vs float32r


def _build_program(act_name="Gelu"):
    import concourse.mybir as mybir
    import concourse.tile as tile
    from concourse import bacc

    f32 = mybir.dt.float32
    f32r = mybir.dt.float32r
    bf16 = mybir.dt.bfloat16
    mm1dt = bf16 if MM1_BF16 else f32r
    Act = mybir.ActivationFunctionType
    Alu = mybir.AluOpType

    nc = bacc.Bacc("TRN2", num_devices=E)
    xt_d = nc.dram_tensor("xt", [D, C_PER], mm1dt, kind="ExternalInput")
    w1_d = nc.dram_tensor("w1r", [FT, P, DT, P], mm1dt, kind="ExternalInput")
    w2_d = nc.dram_tensor("w2b", [FF, D], bf16, kind="ExternalInput")
    sw_d = nc.dram_tensor("swt", [P, NCT], f32, kind="ExternalInput")
    y_d = nc.dram_tensor("y", [C_PER, D], f32, kind="ExternalOutput")

    with tile.TileContext(nc) as tc:
        with tc.tile_pool(name="sb", bufs=1) as sb, \
             tc.tile_pool(name="w1p", bufs=8) as w1p, \
             tc.tile_pool(name="yop", bufs=3) as yop, \
             tc.tile_pool(name="psh", bufs=4, space="PSUM") as psh, \
             tc.tile_pool(name="psy", bufs=4, space="PSUM") as psy:
            cpool = xtp = w2p = hp = sb

            w2t = [None] * FT
            xt = {}

            # HAM warm-up: the PE clock gate opens only after ~3.4us of
            # sustained tensor activity. Real matmuls can't start until
            # the first xT/w1 DMAs land (~3us after the framework
            # preamble), so burn that window on dummy matmuls over a
            # memset tile — by the time real data arrives the PE is at
            # full clock instead of 1.2 GHz.
            wz = sb.tile([P, CB], mm1dt, tag="wz", name="wz")
            nc.gpsimd.memset(wz, 0)
            pwarm = psy.tile([P, CB], f32, tag="psy", name="pwarm")
            for _ in range(NWARM):
                nc.tensor.matmul(pwarm, wz[:, 0:P], wz, start=True,
                                 stop=True)

            def alloc_xt(d, pair):
                t = xtp.tile([P, 2 * CB], mm1dt, tag=f"xt_{d}_{pair}",
                             name=f"xt_{d}_{pair}")
                xt[(d, pair)] = t
                return t

            def load_xt_full(d, pair, eng):
                # one tile spans BOTH c-blocks of a pair: their columns
                # are adjacent in C, so each partition line is one 2 KB
                # contiguous DMA line
                t = alloc_xt(d, pair)
                eng.dma_start(
                    out=t,
                    in_=xt_d.ap()[d * P:(d + 1) * P,
                                  pair * 2 * CB:(pair + 1) * 2 * CB])

            def load_xt_half(d, pair, half, eng):
                t = xt.get((d, pair))
                if t is None:
                    t = alloc_xt(d, pair)
                c0 = pair * 2 * CB + half * CB
                eng.dma_start(
                    out=t[:, half * CB:(half + 1) * CB],
                    in_=xt_d.ap()[d * P:(d + 1) * P, c0:c0 + CB])

            # Startup: both hardware DGE rings (SP=sync, Act=scalar)
            # stage the first f-tile group's minimal working set first:
            # ch=0 column halves of every d-tile plus the two w1 halves,
            # interleaved so the d-order of arrival matches the d-order
            # of the first matmul group. ch=1 halves follow.
            t0 = alloc_xt(0, 0)
            nc.scalar.dma_start(out=t0[:, 0:CB], in_=xt_d.ap()[0:P, 0:CB])
            w1t0a = w1p.tile([P, DT // 2, P], mm1dt, tag="w1a", name="w1a")
            nc.sync.dma_start(out=w1t0a, in_=w1_d.ap()[0, :, 0:DT // 2])
            w1t0b = w1p.tile([P, DT // 2, P], mm1dt, tag="w1b", name="w1b")
            nc.scalar.dma_start(out=w1t0b, in_=w1_d.ap()[0, :, DT // 2:DT])
            for d in range(1, DT):
                load_xt_half(d, 0, 0, nc.sync if d % 2 else nc.scalar)
            # late-deadline ch=1 halves of d4..d7 ride the GpSimd
            # software-DGE queue — a third SDMA queue whose transfers
            # run concurrently with both HW rings, cutting ~512 KB from
            # each ring's congested startup window
            for d in range(4, DT):
                load_xt_half(d, 0, 1, nc.gpsimd)
            for d in range(4):
                load_xt_half(d, 0, 1, nc.scalar if d % 2 == 0 else nc.sync)
            swt = cpool.tile([P, NCT], f32, tag="swt")
            nc.scalar.dma_start(out=swt, in_=sw_d.ap())

            def load_w2(ft):
                # full-width tiles: 2 KB DMA lines per partition (the
                # d-half split had 1 KB lines, doubling packet count on
                # the DGE queue)
                t = w2p.tile([P, D], bf16, tag=f"w2_{ft}",
                             name=f"w2_{ft}")
                nc.sync.dma_start(
                    out=t, in_=w2_d.ap()[ft * P:(ft + 1) * P, :])
                w2t[ft] = t

            # c-blocks are processed in PAIRS for mm1: each streamed w1
            # tile feeds 16 matmuls (two 512-col groups) instead of 8,
            # halving both the w1 consumption cadence and total w1
            # re-streaming (32 MB -> 16 MB).
            ht = {}
            ndrain = 0
            for pair in range(NCB // 2):
                cbs = (2 * pair, 2 * pair + 1)
                for ft in range(FT):
                    if pair == 0 and ft == 0:
                        def w1ap(d, a=w1t0a, b=w1t0b):
                            t = a if d < DT // 2 else b
                            return t[:, d % (DT // 2), :]
                    else:
                        w1t = w1p.tile([P, DT, P], mm1dt, tag="w1",
                                       name="w1t")
                        nc.sync.dma_start(out=w1t, in_=w1_d.ap()[ft])

                        def w1ap(d, t=w1t):
                            return t[:, d, :]
                    if pair == 0:
                        load_w2(ft)
                    for ch, cb in enumerate(cbs):
                        hps = psh.tile([P, CB], f32, tag="psh", name="hps")
                        for d in range(DT):
                            nc.tensor.matmul(
                                hps, w1ap(d),
                                xt[(d, pair)][:, ch * CB:(ch + 1) * CB],
                                start=(d == 0), stop=(d == DT - 1))
                        h_t = hp.tile([P, CB], bf16, tag=f"h_{ch}_{ft}",
                                      name=f"h_{ch}_{ft}")
                        nc.scalar.activation(h_t, hps,
                                             getattr(Act, act_name))
                        ht[(ch, ft)] = h_t
                # mm2 per c-block of the pair: db-outer so the db=1
                # groups run last — their w2 half (pair 0) streams
                # while the db=0 groups compute
                for ch, cb in enumerate(cbs):
                    for db in range(2):
                        d0 = db * (D // 2)
                        for ct in range(CB // P):
                            g = cb * (CB // P) + ct
                            last = (cb == NCB - 1 and db == 1
                                    and ct == CB // P - 1)
                            if last:
                                # split the final accumulation so half A
                                # drains behind half B's matmuls and only
                                # one DVE merge + the DMA stay exposed
                                ypsA = psy.tile([P, D // 2], f32,
                                                tag="psy", name="ypsA")
                                for ft in range(FT // 2):
                                    nc.tensor.matmul(
                                        ypsA,
                                        ht[(ch, ft)][:, ct * P:(ct + 1) * P],
                                        w2t[ft][:, d0:d0 + D // 2],
                                        start=(ft == 0),
                                        stop=(ft == FT // 2 - 1))
                                yoA = yop.tile([P, D // 2], f32, tag="yo",
                                               name="yoA")
                                nc.scalar.activation(
                                    yoA, ypsA, Act.Copy,
                                    scale=swt[:, g:g + 1])
                                ypsB = psy.tile([P, D // 2], f32,
                                                tag="psy", name="ypsB")
                                for ft in range(FT // 2, FT):
                                    nc.tensor.matmul(
                                        ypsB,
                                        ht[(ch, ft)][:, ct * P:(ct + 1) * P],
                                        w2t[ft][:, d0:d0 + D // 2],
                                        start=(ft == FT // 2),
                                        stop=(ft == FT - 1))
                                yo = yop.tile([P, D // 2], f32, tag="yo",
                                              name="yoS")
                                # quarter-wise merge: each DVE quarter's
                                # output DMA launches while the next
                                # quarter is still merging, alternating
                                # DGE rings
                                quart = D // 8
                                for q in range(4):
                                    q0 = q * quart
                                    nc.vector.scalar_tensor_tensor(
                                        yo[:, q0:q0 + quart],
                                        ypsB[:, q0:q0 + quart],
                                        swt[:, g:g + 1],
                                        yoA[:, q0:q0 + quart],
                                        Alu.mult, Alu.add)
                                    eng = nc.sync if q % 2 == 0 else nc.scalar
                                    eng.dma_start(
                                        out=y_d.ap()[
                                            g * P:(g + 1) * P,
                                            d0 + q0:d0 + q0 + quart],
                                        in_=yo[:, q0:q0 + quart])
                                continue
                            yps = psy.tile([P, D // 2], f32, tag="psy",
                                           name="yps")
                            for ft in range(FT):
                                nc.tensor.matmul(
                                    yps,
                                    ht[(ch, ft)][:, ct * P:(ct + 1) * P],
                                    w2t[ft][:, d0:d0 + D // 2],
                                    start=(ft == 0), stop=(ft == FT - 1))
                            yo = yop.tile([P, D // 2], f32, tag="yo",
                                          name="yo")
                            nc.scalar.activation(yo, yps,
                                                 Act.Copy,
                                                 scale=swt[:, g:g + 1])
                            # alternate output DMAs across the two DGE
                            # rings
                            eng = nc.sync if ndrain % 2 == 0 else nc.scalar
                            ndrain += 1
                            eng.dma_start(
                                out=y_d.ap()[g * P:(g + 1) * P,
                                             d0:d0 + D // 2],
                                in_=yo)
                            # prefetch the next pair's xT tiles inside
                            # the long mm2 phase (scalar ring — sync
                            # carries the w1/w2 streams)
                            if (ch == 0 and db == 0 and ct == 0
                                    and pair + 1 < NCB // 2):
                                for d in range(DT):
                                    load_xt_full(d, pair + 1, nc.scalar)
    nc.compile()
    return nc


def _get_program():
    if "nc" not in _cache:
        _cache["nc"] = _build_program()
    return _cache["nc"]


def _routing(xf, router_w):
    """Replicate the reference gating bit-exactly where it matters.

    Returns (rev, sw, sort_idx). The top-k *selection* must match the
    reference exactly (it is discrete); we therefore compute the router
    logits with jax when available, mirroring reference.py. The softmax
    and sort bookkeeping is continuous or exactly replicable in numpy.
    """
    try:
        import jax
        import jax.numpy as jnp

        def _gate():
            logits = jnp.asarray(xf) @ jnp.asarray(router_w).T
            return jax.lax.top_k(logits, TOP_K)

        try:
            cpu = jax.devices("cpu")[0]
            with jax.default_device(cpu):
                tv, ti = _gate()
        except Exception:
            tv, ti = _gate()
        topv = np.asarray(tv, dtype=np.float32)
        topi = np.asarray(ti)
    except Exception:
        logits = xf @ router_w.T
        # top-2 with jax tie-breaking (lower index wins)
        i0 = np.argmax(logits, axis=-1)
        v0 = np.take_along_axis(logits, i0[:, None], axis=-1)[:, 0]
        masked = logits.copy()
        np.put_along_axis(masked, i0[:, None], -np.inf, axis=-1)
        i1 = np.argmax(masked, axis=-1)
        v1 = np.take_along_axis(logits, i1[:, None], axis=-1)[:, 0]
        topi = np.stack([i0, i1], axis=-1)
        topv = np.stack([v0, v1], axis=-1).astype(np.float32)

    # softmax over the two gate logits, float32
    m = topv.max(axis=-1, keepdims=True)
    e = np.exp(topv - m, dtype=np.float32)
    topw = (e / e.sum(axis=-1, keepdims=True)).astype(np.float32)

    idx_flat = topi.reshape(-1)
    w_flat = topw.reshape(-1)
    # stable argsort of integer keys is uniquely determined by the keys
    sort_idx = np.argsort(idx_flat, kind="stable")
    src = np.repeat(np.arange(N), TOP_K)
    rev = src[sort_idx]
    sw = w_flat[sort_idx]
    return rev, sw, sort_idx


def _ensure_axon_hooks():
    """Make `antenv.axon_hooks` importable so run_bass_kernel_spmd's
    trace path degrades gracefully (or works, if the axon boot shim is
    available) instead of crashing on ImportError."""
    try:
        import antenv.axon_hooks  # noqa: F401
        return
    except ImportError:
        pass
    import sys
    import types
    mod = types.ModuleType("antenv.axon_hooks")
    state = {"hook": None}
    mod.set_axon_ntff_profile_hook = lambda h: state.update(hook=h)
    mod.get_axon_ntff_profile_hook = lambda: state["hook"]
    try:
        import antenv
        sys.modules["antenv.axon_hooks"] = mod
        antenv.axon_hooks = mod
    except ImportError:
        return
    try:
        from trn_agent_boot.trn_boot import _ntff_profile_via_ctypes
        h = _ntff_profile_via_ctypes("/opt/axon/libaxon_pjrt.so")
        if h is not None:
            mod.set_axon_ntff_profile_hook(h)
            import concourse.bass_utils as bu
            bu.upload_artifacts = lambda tmpdir: "local://" + str(tmpdir)
    except Exception:
        pass


def kernel(x, router_w, w1, w2):
    import ml_dtypes
    from concourse import bass_utils
    _ensure_axon_hooks()

    xf = np.ascontiguousarray(x.reshape(-1, D), dtype=np.float32)
    rev, sw, sort_idx = _routing(xf, router_w)

    nc = _get_program()

    in_maps = []
    for e in range(E):
        rows = rev[e * C_PER:(e + 1) * C_PER]
        xct = np.ascontiguousarray(xf[rows].T)
        w1r = np.ascontiguousarray(
            w1[e].reshape(DT, P, FT, P).transpose(2, 1, 0, 3))
        if MM1_BF16:
            xct = xct.astype(ml_dtypes.bfloat16)
            w1r = w1r.astype(ml_dtypes.bfloat16)
        w2b = np.ascontiguousarray(w2[e].astype(ml_dtypes.bfloat16))
        swt = np.ascontiguousarray(
            sw[e * C_PER:(e + 1) * C_PER].reshape(NCT, P).T)
        in_maps.append({"xt": xct, "w1r": w1r, "w2b": w2b, "swt": swt})

    r = bass_utils.run_bass_kernel_spmd(nc, in_maps, core_ids=list(range(E)))
    _cache["last_result"] = r

    y_sorted = np.empty((NE, D), dtype=np.float32)
    for e in range(E):
        y_sorted[e * C_PER:(e + 1) * C_PER] = r.results[e]["y"]

    # invert the permutation and combine the top-2 contributions
    y_expanded = np.empty_like(y_sorted)
    y_expanded[sort_idx] = y_sorted
    out = y_expanded.reshape(N, TOP_K, D).sum(axis=1)
    return out.reshape(B, T, D)
